# revision 20
# baseline (speedup 1.0000x reference)
"""Trainium2 Bass kernel for nn_FLB_Attention_Layer (gated fusion + additive
attention over 3 tokens + output projection, with residuals).

Data-parallel over batch B=4096 across 8 NeuronCores (512 samples/core,
weights replicated). The device computes the attention-layer output
(without residual) in feature-major layout; the host adds the residual
and transposes back to batch-major.

Numerics: all six D x D GEMMs run as fp8e4 (e4m3) DoubleRow matmuls
(2 fp8 weights per PE cell, 256-deep contraction per instruction, which
doubles throughput over bf16: measured 216 ns per [256 x 128 x 512] MM).
Weights are host-prescaled by 16, transposed to W.T, and pre-tiled to
the SBUF layout [p=in%128, half, k=in//128, out] so every DMA descriptor
moves a 16 KB contiguous run per partition. Tokens are host-prescaled by
16 and pre-tiled the same way. QKV psums carry 256x the true scale;
activation-engine evictions fold the rescale into their scale argument.

Additive attention per head: T = tanh(q_i + k_j) in fp8, scores via a
broadcast matmul (lhsT = v_a replicated across 128 columns, x64 scale)
so softmax operates on partition-replicated tiles; exp on ACT (batched
[P,3,512] tiles straight from a 3-bank PSUM tile), denominators via the
fast approximate reciprocal, weighted sum on DVE. Per-head emission is
software-pipelined: PE runs head h's QKV chains, then head h-1's score
matmuls, so ACT/DVE latency hides under the matmul stream.
"""

import numpy as np

P = 128
D = 2048
H = 16
DH = 128
KT = D // P  # 16
B = 4096
N_CORES = 8
B_C = B // N_CORES  # 512

SW = 16.0  # weight prescale (host)
ST = 16.0  # token prescale (host)
SV = 64.0  # v_a prescale (host)

_compiled = {}


def _build(b_c=B_C, d=D, h=H):
    import concourse.bass as bass
    import concourse.mybir as mybir
    import concourse.tile as tile
    from contextlib import ExitStack
    from concourse import bacc

    f32 = mybir.dt.float32
    f16 = mybir.dt.float16
    f8 = mybir.dt.float8e4
    AF = mybir.ActivationFunctionType
    DR = mybir.MatmulPerfMode.DoubleRow

    kt = d // P
    nh = h
    WQ = kt * 512  # elements per weight quarter, per partition

    nc = bacc.Bacc(None, target_bir_lowering=False, debug=False)

    toks = [
        nc.declare_dram_parameter(f"tok{t}", [P, kt * b_c], f16, isOutput=False)
        for t in range(2)
    ]
    fdbk8 = nc.declare_dram_parameter("fdbk8", [P, kt * b_c], f8, isOutput=False)
    wgl = nc.declare_dram_parameter("wgl", [P, 4 * WQ], f8, isOutput=False)
    wgx = nc.declare_dram_parameter("wgx", [P, 4 * WQ], f8, isOutput=False)
    wq = nc.declare_dram_parameter("wq", [P, 4 * WQ], f8, isOutput=False)
    wk = nc.declare_dram_parameter("wk", [P, 4 * WQ], f8, isOutput=False)
    wv = nc.declare_dram_parameter("wv", [P, 4 * WQ], f8, isOutput=False)
    wo = nc.declare_dram_parameter("wo", [P, 4 * WQ], f8, isOutput=False)
    bglT = nc.declare_dram_parameter("bglT", [P, kt], f32, isOutput=False)
    bgxT = nc.declare_dram_parameter("bgxT", [P, kt], f32, isOutput=False)
    vab = nc.declare_dram_parameter("vab", [P, nh * P], f8, isOutput=False)
    oall = nc.declare_dram_parameter("oall", [P, 3 * kt * b_c], f16, isOutput=True)

    with tile.TileContext(nc) as tc:
        with ExitStack() as ctx:
            const = ctx.enter_context(tc.tile_pool(name="const", bufs=1))
            ptok = ctx.enter_context(tc.tile_pool(name="ptok", bufs=1))
            pw = ctx.enter_context(tc.tile_pool(name="pw", bufs=6))
            pg = ctx.enter_context(tc.tile_pool(name="pg", bufs=2))
            pqk = ctx.enter_context(tc.tile_pool(name="pqk", bufs=2))
            pv = ctx.enter_context(tc.tile_pool(name="pv", bufs=2))
            pT = ctx.enter_context(tc.tile_pool(name="pT", bufs=2))
            pE = ctx.enter_context(tc.tile_pool(name="pE", bufs=3))
            psm = ctx.enter_context(tc.tile_pool(name="psm", bufs=1))
            pout = ctx.enter_context(tc.tile_pool(name="pout", bufs=3))
            ps_mm = ctx.enter_context(tc.tile_pool(name="ps_mm", bufs=2, space="PSUM"))
            ps_s = ctx.enter_context(tc.tile_pool(name="ps_s", bufs=2, space="PSUM"))

            wdum = const.tile([P, P], f8)
            xdum = const.tile([P, b_c], f8)
            nc.vector.memset(wdum[:], 0.0)
            nc.vector.memset(xdum[:], 0.0)

            def pe_warm(n):
                for _ in range(n):
                    psD = ps_mm.tile([P, b_c], f32, tag="mm")
                    nc.tensor.matmul(psD[:], wdum[:], xdum[:], start=True, stop=True)

            pe_warm(70)

            bgl_t = const.tile([P, kt], f32)
            bgx_t = const.tile([P, kt], f32)
            va8 = const.tile([P, nh, P], f8)
            # master x/lat tokens (16x true, f16) and fp8 mirror for matmul
            # rhs; token-major free layout [tok, k, b] keeps DMA contiguous
            tokF = ptok.tile([P, 2, kt, b_c], f16)
            tok8 = ptok.tile([P, 3, kt, b_c], f8)
            attT = ptok.tile([P, 3, kt, b_c], f8)
            nc.scalar.dma_start(tok8[:, 2, :, :], fdbk8[:])
            nc.scalar.dma_start(bgl_t[:], bglT[:])
            for t in (1, 0):
                nc.scalar.dma_start(tokF[:, t, :, :], toks[t][:])
            nc.scalar.dma_start(bgx_t[:], bgxT[:])
            nc.scalar.dma_start(va8[:], vab[:])

            def load_w(Wd, q):
                wh = pw.tile([P, kt, 512], f8, tag="wh")
                nc.sync.dma_start(wh[:], Wd[:, q * WQ : (q + 1) * WQ])
                return wh

            def gemm_chain(psum_ap, wh, oc, rhs_kb):
                """8 DoubleRow matmuls contracting all of D."""
                for kp in range(8):
                    nc.tensor.matmul(
                        psum_ap,
                        wh[:, 2 * kp : 2 * kp + 2, oc * P : (oc + 1) * P],
                        rhs_kb[:, 2 * kp : 2 * kp + 2, :],
                        start=(kp == 0),
                        stop=(kp == 7),
                        perf_mode=DR,
                    )

            # ---- gated fusion ----
            # G_L = sigmoid(fdbk @ WgL.T + bgL); lat' = lat * G_L
            # G_X = sigmoid(lat' @ WgX.T + bgX); x'   = x * G_X
            for Wd, bg_t, src_t, dst_t in ((wgl, bgl_t, 2, 1), (wgx, bgx_t, 1, 0)):
                for q in range(4):
                    wh = load_w(Wd, q)
                    for oc in range(4):
                        ot = q * 4 + oc
                        pgp = ps_mm.tile([P, b_c], f32, tag="mm")
                        gemm_chain(pgp[:], wh, oc, tok8[:, src_t, :, :])
                        gate = pg.tile([P, b_c], f16, tag="gate")
                        nc.scalar.activation(
                            gate[:],
                            pgp[:],
                            AF.Sigmoid,
                            bias=bg_t[:, ot : ot + 1],
                            scale=1.0 / (SW * ST),
                        )
                        nc.vector.tensor_mul(
                            tok8[:, dst_t, ot, :],
                            tokF[:, dst_t, ot, :],
                            gate[:],
                        )

            # ---- QKV + additive attention, 2 groups of 8 heads ----
            def emit_scores(hh, T8):
                Es = []
                for i in range(3):
                    E = pE.tile([P, 3, b_c], f16, tag="E", name=f"E{i}")
                    sps = ps_s.tile([P, 3, b_c], f32, tag="sc")
                    for j in range(3):
                        nc.tensor.matmul(
                            sps[:, j, :],
                            va8[:, hh, :],
                            T8[:, 3 * i + j, :],
                            start=True,
                            stop=True,
                        )
                    nc.scalar.activation(E[:], sps[:], AF.Exp, scale=1.0 / SV)
                    Es.append(E)
                return Es

            def emit_softmax(hh, Es, vh):
                for i in range(3):
                    E = Es[i]
                    den = psm.tile([P, b_c], f32, tag="den")
                    nc.vector.tensor_add(den[:], E[:, 0, :], E[:, 1, :])
                    nc.vector.tensor_add(den[:], den[:], E[:, 2, :])
                    rden = psm.tile([P, b_c], f32, tag="rden")
                    nc.vector.reciprocal_approx_fast(rden[:], den[:])
                    acc = psm.tile([P, b_c], f16, tag="acc")
                    tmp = psm.tile([P, b_c], f16, tag="tmp")
                    nc.vector.tensor_mul(acc[:], E[:, 0, :], vh[:, 0, :])
                    nc.vector.tensor_mul(tmp[:], E[:, 1, :], vh[:, 1, :])
                    nc.vector.tensor_add(acc[:], acc[:], tmp[:])
                    nc.vector.tensor_mul(tmp[:], E[:, 2, :], vh[:, 2, :])
                    nc.vector.tensor_add(acc[:], acc[:], tmp[:])
                    nc.vector.tensor_mul(attT[:, i, hh, :], acc[:], rden[:])

            pend = None
            for sg in range(4):
                whq = load_w(wq, sg)
                whk = load_w(wk, sg)
                whv = load_w(wv, sg)
                for hg in range(4):
                    hh = sg * 4 + hg
                    qf = pqk.tile([P, 3, b_c], f16, tag="qf")
                    kf = pqk.tile([P, 3, b_c], f16, tag="kf")
                    vh = pv.tile([P, 3, b_c], f16, tag="vh")
                    for wh_, dst in ((whq, qf), (whk, kf), (whv, vh)):
                        for t in range(3):
                            pp = ps_mm.tile([P, b_c], f32, tag="mm")
                            gemm_chain(pp[:], wh_, hg, tok8[:, t, :, :])
                            # keep ACT free near the chain tail so exp of the
                            # previous head's scores starts immediately
                            if dst is vh and t >= 1:
                                nc.vector.tensor_scalar_mul(
                                    dst[:, t, :], pp[:], 1.0 / SW
                                )
                            else:
                                nc.scalar.activation(
                                    dst[:, t, :], pp[:], AF.Copy, scale=1.0 / SW
                                )
                    if pend is not None:
                        ph, pT8, pvh = pend
                        pEs = emit_scores(ph, pT8)
                    T8 = pT.tile([P, 9, b_c], f8, tag="T8")
                    for i in range(3):
                        pre3 = pg.tile([P, 3, b_c], f16, tag="pre")
                        for j in range(3):
                            nc.vector.tensor_add(
                                pre3[:, j, :], qf[:, i, :], kf[:, j, :]
                            )
                        nc.scalar.activation(
                            T8[:, 3 * i : 3 * i + 3, :], pre3[:], AF.Tanh,
                            scale=1.0 / ST,
                        )
                    if pend is not None:
                        emit_softmax(ph, pEs, pvh)
                    pend = (hh, T8, vh)
            ph, pT8, pvh = pend
            emit_softmax(ph, emit_scores(ph, pT8), pvh)

            # ---- output projection (no residual; host adds it) ----
            def wo_mm(po3, who, oc, kp, t, stop):
                nc.tensor.matmul(
                    po3[:, t, :],
                    who[:, 2 * kp : 2 * kp + 2, oc * P : (oc + 1) * P],
                    attT[:, t, 2 * kp : 2 * kp + 2, :],
                    start=(kp == 0),
                    stop=stop,
                    perf_mode=DR,
                )

            oall_v = oall[:].rearrange("p (t k b) -> p t k b", t=3, b=b_c)

            def wo_evict(po3, ot):
                of3 = pout.tile([P, 3, b_c], f16, tag="of")
                nc.scalar.activation(of3[:], po3[:], AF.Copy, scale=1.0 / SW)
                nc.scalar.dma_start(oall_v[:, :, ot, :], of3[:])

            who0 = load_w(wo, 0)
            # prefill: ot0/ot1 contract heads 0-13 while the last heads'
            # softmax drains; the head-14/15 pair (kp=7) lands afterwards
            po3a = ps_s.tile([P, 3, b_c], f32, tag="sc")
            po3b = ps_s.tile([P, 3, b_c], f32, tag="sc")
            for po3_, oc_ in ((po3a, 0), (po3b, 1)):
                for kp in range(7):
                    for t in range(3):
                        wo_mm(po3_, who0, oc_, kp, t, False)
            pe_warm(36)
            first_grp = True
            for po3_, oc_ in ((po3a, 0), (po3b, 1)):
                for t in range(3):
                    wo_mm(po3_, who0, oc_, 7, t, True)
                    # the next attT(i) lands ~4.3us later; keep PE warm
                    if first_grp and t < 2:
                        pe_warm(16)
                wo_evict(po3_, oc_)
                first_grp = False
            for q in range(4):
                who = who0 if q == 0 else load_w(wo, q)
                for oc in range(2 if q == 0 else 0, 4):
                    ot = q * 4 + oc
                    if ot % 2 == 0:
                        po3 = ps_s.tile([P, 3, b_c], f32, tag="sc")
                        for kp in range(8):
                            for t in range(3):
                                wo_mm(po3, who, oc, kp, t, kp == 7)
                        wo_evict(po3, ot)
                    else:
                        # odd tiles run through the idle single-bank pool so
                        # the 3-bank score-psum rotation gets 2x the slack
                        of3 = pout.tile([P, 3, b_c], f16, tag="of")
                        for t in range(3):
                            po1 = ps_mm.tile([P, b_c], f32, tag="mm")
                            for kp in range(8):
                                nc.tensor.matmul(
                                    po1[:],
                                    who[:, 2 * kp : 2 * kp + 2, oc * P : (oc + 1) * P],
                                    attT[:, t, 2 * kp : 2 * kp + 2, :],
                                    start=(kp == 0),
                                    stop=(kp == 7),
                                    perf_mode=DR,
                                )
                            nc.scalar.activation(
                                of3[:, t, :], po1[:], AF.Copy, scale=1.0 / SW
                            )
                        nc.scalar.dma_start(oall_v[:, :, ot, :], of3[:])

    nc.compile()
    return nc


def _get_nc():
    key = "full"
    if key not in _compiled:
        _compiled[key] = _build()
    return _compiled[key]


def _tile_rows(a):
    """[D, N] -> [128, (D//128)*N] with row d at [d%128, (d//128)*N:...]."""
    dd, n = a.shape
    return np.ascontiguousarray(
        a.reshape(dd // P, P, n).transpose(1, 0, 2).reshape(P, (dd // P) * n)
    )


def kernel(
    x_token,
    lat_token,
    fdbk_token,
    W_gate_L,
    b_gate_L,
    W_gate_X,
    b_gate_X,
    W_q,
    W_k,
    W_v,
    W_o,
    v_a,
):
    import ml_dtypes
    from concourse.bass_utils import run_bass_kernel_spmd

    nc = _get_nc()
    f32 = np.float32
    e4 = ml_dtypes.float8_e4m3

    def prep_w(W):
        wt = np.asarray(W, f32).T * SW  # [in, out] = W.T
        wt = np.clip(wt, -240.0, 240.0)
        # -> [p=in%128, (quarter, k=in//128, out%512)]
        a = wt.reshape(KT, P, 4, 512).transpose(1, 2, 0, 3)
        return np.ascontiguousarray(a.reshape(P, 4 * KT * 512)).astype(e4)

    w8 = {
        "wgl": prep_w(W_gate_L),
        "wgx": prep_w(W_gate_X),
        "wq": prep_w(W_q),
        "wk": prep_w(W_k),
        "wv": prep_w(W_v),
        "wo": prep_w(W_o),
    }
    bglT = np.ascontiguousarray(np.asarray(b_gate_L, f32).reshape(KT, P).T)
    bgxT = np.ascontiguousarray(np.asarray(b_gate_X, f32).reshape(KT, P).T)
    va = np.asarray(v_a, f32).reshape(H, DH).T * SV  # [DH, H]
    vab = np.ascontiguousarray(
        np.repeat(va[:, :, None], P, axis=2).reshape(DH, H * P)
    ).astype(e4)

    tok_full = [
        np.asarray(t, f32).reshape(B, D) for t in (x_token, lat_token, fdbk_token)
    ]
    tokT16 = [(tok_full[t].T * ST).astype(np.float16) for t in range(2)]
    fdbkT8 = np.clip(tok_full[2].T * ST, -240.0, 240.0).astype(e4)

    in_maps = []
    for c in range(N_CORES):
        s = slice(c * B_C, (c + 1) * B_C)
        m = {f"tok{t}": _tile_rows(tokT16[t][:, s]) for t in range(2)}
        m["fdbk8"] = _tile_rows(fdbkT8[:, s])
        m.update(w8)
        m.update({"bglT": bglT, "bgxT": bgxT, "vab": vab})
        in_maps.append(m)

    res = run_bass_kernel_spmd(nc, in_maps, list(range(N_CORES))).results

    out = []
    for t in range(3):
        cores = []
        for c in range(N_CORES):
            r = res[c]["oall"].reshape(P, 3, KT, B_C)[:, t]
            cores.append(
                np.ascontiguousarray(r).transpose(1, 0, 2).reshape(D, B_C)
            )
        full = np.concatenate(cores, axis=1)  # [D, B]
        o = full.T.astype(f32) / SW + tok_full[t]
        out.append(o.reshape(B, 1, D))
    return tuple(out)


# revision 21
# speedup vs baseline: 1.0060x; 1.0060x over previous
"""Trainium2 Bass kernel for nn_FLB_Attention_Layer (gated fusion + additive
attention over 3 tokens + output projection, with residuals).

Data-parallel over batch B=4096 across 8 NeuronCores (512 samples/core,
weights replicated). The device computes the attention-layer output
(without residual) in feature-major layout; the host adds the residual
and transposes back to batch-major.

Numerics: all six D x D GEMMs run as fp8e4 (e4m3) DoubleRow matmuls
(2 fp8 weights per PE cell, 256-deep contraction per instruction, which
doubles throughput over bf16: measured 216 ns per [256 x 128 x 512] MM).
Weights are host-prescaled by 16, transposed to W.T, and pre-tiled to
the SBUF layout [p=in%128, half, k=in//128, out] so every DMA descriptor
moves a 16 KB contiguous run per partition. Tokens are host-prescaled by
16 and pre-tiled the same way. QKV psums carry 256x the true scale;
activation-engine evictions fold the rescale into their scale argument.

Additive attention per head: T = tanh(q_i + k_j) in fp8, scores via a
broadcast matmul (lhsT = v_a replicated across 128 columns, x64 scale)
so softmax operates on partition-replicated tiles; exp on ACT (batched
[P,3,512] tiles straight from a 3-bank PSUM tile), denominators via the
fast approximate reciprocal, weighted sum on DVE. Per-head emission is
software-pipelined: PE runs head h's QKV chains, then head h-1's score
matmuls, so ACT/DVE latency hides under the matmul stream.
"""

import numpy as np

P = 128
D = 2048
H = 16
DH = 128
KT = D // P  # 16
B = 4096
N_CORES = 8
B_C = B // N_CORES  # 512

SW = 16.0  # weight prescale (host)
ST = 16.0  # token prescale (host)
SV = 64.0  # v_a prescale (host)

_compiled = {}


def _build(b_c=B_C, d=D, h=H):
    import concourse.bass as bass
    import concourse.mybir as mybir
    import concourse.tile as tile
    from contextlib import ExitStack
    from concourse import bacc

    f32 = mybir.dt.float32
    f16 = mybir.dt.float16
    f8 = mybir.dt.float8e4
    AF = mybir.ActivationFunctionType
    DR = mybir.MatmulPerfMode.DoubleRow

    kt = d // P
    nh = h
    WQ = kt * 512  # elements per weight quarter, per partition

    nc = bacc.Bacc(None, target_bir_lowering=False, debug=False)

    toks = [
        nc.declare_dram_parameter(f"tok{t}", [P, kt * b_c], f16, isOutput=False)
        for t in range(2)
    ]
    fdbk8 = nc.declare_dram_parameter("fdbk8", [P, kt * b_c], f8, isOutput=False)
    wgl = nc.declare_dram_parameter("wgl", [P, 4 * WQ], f8, isOutput=False)
    wgx = nc.declare_dram_parameter("wgx", [P, 4 * WQ], f8, isOutput=False)
    wq = nc.declare_dram_parameter("wq", [P, 4 * WQ], f8, isOutput=False)
    wk = nc.declare_dram_parameter("wk", [P, 4 * WQ], f8, isOutput=False)
    wv = nc.declare_dram_parameter("wv", [P, 4 * WQ], f8, isOutput=False)
    wo = nc.declare_dram_parameter("wo", [P, 4 * WQ], f8, isOutput=False)
    bglT = nc.declare_dram_parameter("bglT", [P, kt], f32, isOutput=False)
    bgxT = nc.declare_dram_parameter("bgxT", [P, kt], f32, isOutput=False)
    vab = nc.declare_dram_parameter("vab", [P, nh * P], f8, isOutput=False)
    oall = nc.declare_dram_parameter("oall", [P, 3 * kt * b_c], f16, isOutput=True)

    with tile.TileContext(nc) as tc:
        with ExitStack() as ctx:
            const = ctx.enter_context(tc.tile_pool(name="const", bufs=1))
            ptok = ctx.enter_context(tc.tile_pool(name="ptok", bufs=1))
            pw = ctx.enter_context(tc.tile_pool(name="pw", bufs=6))
            pg = ctx.enter_context(tc.tile_pool(name="pg", bufs=2))
            pqk = ctx.enter_context(tc.tile_pool(name="pqk", bufs=2))
            pv = ctx.enter_context(tc.tile_pool(name="pv", bufs=2))
            pT = ctx.enter_context(tc.tile_pool(name="pT", bufs=2))
            pE = ctx.enter_context(tc.tile_pool(name="pE", bufs=3))
            psm = ctx.enter_context(tc.tile_pool(name="psm", bufs=1))
            pout = ctx.enter_context(tc.tile_pool(name="pout", bufs=3))
            ps_mm = ctx.enter_context(tc.tile_pool(name="ps_mm", bufs=2, space="PSUM"))
            ps_s = ctx.enter_context(tc.tile_pool(name="ps_s", bufs=2, space="PSUM"))

            wdum = const.tile([P, P], f8)
            xdum = const.tile([P, b_c], f8)
            nc.vector.memset(wdum[:], 0.0)
            nc.vector.memset(xdum[:], 0.0)

            def pe_warm(n):
                for _ in range(n):
                    psD = ps_mm.tile([P, b_c], f32, tag="mm")
                    nc.tensor.matmul(psD[:], wdum[:], xdum[:], start=True, stop=True)

            pe_warm(30)

            bgl_t = const.tile([P, kt], f32)
            bgx_t = const.tile([P, kt], f32)
            va8 = const.tile([P, nh, P], f8)
            # master x/lat tokens (16x true, f16) and fp8 mirror for matmul
            # rhs; token-major free layout [tok, k, b] keeps DMA contiguous
            tokF = ptok.tile([P, 2, kt, b_c], f16)
            tok8 = ptok.tile([P, 3, kt, b_c], f8)
            attT = ptok.tile([P, 3, kt, b_c], f8)
            nc.scalar.dma_start(tok8[:, 2, :, :], fdbk8[:])
            nc.scalar.dma_start(bgl_t[:], bglT[:])
            for t in (1, 0):
                nc.scalar.dma_start(tokF[:, t, :, :], toks[t][:])
            nc.scalar.dma_start(bgx_t[:], bgxT[:])
            nc.scalar.dma_start(va8[:], vab[:])

            def load_w(Wd, q):
                wh = pw.tile([P, kt, 512], f8, tag="wh")
                nc.sync.dma_start(wh[:], Wd[:, q * WQ : (q + 1) * WQ])
                return wh

            def gemm_chain(psum_ap, wh, oc, rhs_kb):
                """8 DoubleRow matmuls contracting all of D."""
                for kp in range(8):
                    nc.tensor.matmul(
                        psum_ap,
                        wh[:, 2 * kp : 2 * kp + 2, oc * P : (oc + 1) * P],
                        rhs_kb[:, 2 * kp : 2 * kp + 2, :],
                        start=(kp == 0),
                        stop=(kp == 7),
                        perf_mode=DR,
                    )

            # ---- gated fusion ----
            # G_L = sigmoid(fdbk @ WgL.T + bgL); lat' = lat * G_L
            # G_X = sigmoid(lat' @ WgX.T + bgX); x'   = x * G_X
            for Wd, bg_t, src_t, dst_t in ((wgl, bgl_t, 2, 1), (wgx, bgx_t, 1, 0)):
                for q in range(4):
                    wh = load_w(Wd, q)
                    for oc in range(4):
                        ot = q * 4 + oc
                        pgp = ps_mm.tile([P, b_c], f32, tag="mm")
                        gemm_chain(pgp[:], wh, oc, tok8[:, src_t, :, :])
                        gate = pg.tile([P, b_c], f16, tag="gate")
                        nc.scalar.activation(
                            gate[:],
                            pgp[:],
                            AF.Sigmoid,
                            bias=bg_t[:, ot : ot + 1],
                            scale=1.0 / (SW * ST),
                        )
                        nc.vector.tensor_mul(
                            tok8[:, dst_t, ot, :],
                            tokF[:, dst_t, ot, :],
                            gate[:],
                        )

            # ---- QKV + additive attention, 2 groups of 8 heads ----
            def emit_scores(hh, T8):
                Es = []
                for i in range(3):
                    E = pE.tile([P, 3, b_c], f16, tag="E", name=f"E{i}")
                    sps = ps_s.tile([P, 3, b_c], f32, tag="sc")
                    for j in range(3):
                        nc.tensor.matmul(
                            sps[:, j, :],
                            va8[:, hh, :],
                            T8[:, 3 * i + j, :],
                            start=True,
                            stop=True,
                        )
                    nc.scalar.activation(E[:], sps[:], AF.Exp, scale=1.0 / SV)
                    Es.append(E)
                return Es

            def emit_softmax(hh, Es, vh):
                for i in range(3):
                    E = Es[i]
                    den = psm.tile([P, b_c], f32, tag="den")
                    nc.vector.tensor_add(den[:], E[:, 0, :], E[:, 1, :])
                    nc.vector.tensor_add(den[:], den[:], E[:, 2, :])
                    rden = psm.tile([P, b_c], f32, tag="rden")
                    nc.vector.reciprocal_approx_fast(rden[:], den[:])
                    acc = psm.tile([P, b_c], f16, tag="acc")
                    tmp = psm.tile([P, b_c], f16, tag="tmp")
                    nc.vector.tensor_mul(acc[:], E[:, 0, :], vh[:, 0, :])
                    nc.vector.tensor_mul(tmp[:], E[:, 1, :], vh[:, 1, :])
                    nc.vector.tensor_add(acc[:], acc[:], tmp[:])
                    nc.vector.tensor_mul(tmp[:], E[:, 2, :], vh[:, 2, :])
                    nc.vector.tensor_add(acc[:], acc[:], tmp[:])
                    nc.vector.tensor_mul(attT[:, i, hh, :], acc[:], rden[:])

            pend = None
            for sg in range(4):
                whq = load_w(wq, sg)
                whk = load_w(wk, sg)
                whv = load_w(wv, sg)
                for hg in range(4):
                    hh = sg * 4 + hg
                    qf = pqk.tile([P, 3, b_c], f16, tag="qf")
                    kf = pqk.tile([P, 3, b_c], f16, tag="kf")
                    vh = pv.tile([P, 3, b_c], f16, tag="vh")
                    for wh_, dst in ((whq, qf), (whk, kf), (whv, vh)):
                        for t in range(3):
                            pp = ps_mm.tile([P, b_c], f32, tag="mm")
                            gemm_chain(pp[:], wh_, hg, tok8[:, t, :, :])
                            # keep ACT free near the chain tail so exp of the
                            # previous head's scores starts immediately
                            if dst is vh and t >= 1:
                                nc.vector.tensor_scalar_mul(
                                    dst[:, t, :], pp[:], 1.0 / SW
                                )
                            else:
                                nc.scalar.activation(
                                    dst[:, t, :], pp[:], AF.Copy, scale=1.0 / SW
                                )
                    if pend is not None:
                        ph, pT8, pvh = pend
                        pEs = emit_scores(ph, pT8)
                    T8 = pT.tile([P, 9, b_c], f8, tag="T8")
                    for i in range(3):
                        pre3 = pg.tile([P, 3, b_c], f16, tag="pre")
                        for j in range(3):
                            nc.vector.tensor_add(
                                pre3[:, j, :], qf[:, i, :], kf[:, j, :]
                            )
                        nc.scalar.activation(
                            T8[:, 3 * i : 3 * i + 3, :], pre3[:], AF.Tanh,
                            scale=1.0 / ST,
                        )
                    if pend is not None:
                        emit_softmax(ph, pEs, pvh)
                    pend = (hh, T8, vh)
            ph, pT8, pvh = pend
            emit_softmax(ph, emit_scores(ph, pT8), pvh)

            # ---- output projection (no residual; host adds it) ----
            def wo_mm(po3, who, oc, kp, t, stop):
                nc.tensor.matmul(
                    po3[:, t, :],
                    who[:, 2 * kp : 2 * kp + 2, oc * P : (oc + 1) * P],
                    attT[:, t, 2 * kp : 2 * kp + 2, :],
                    start=(kp == 0),
                    stop=stop,
                    perf_mode=DR,
                )

            oall_v = oall[:].rearrange("p (t k b) -> p t k b", t=3, b=b_c)

            def wo_evict(po3, ot):
                of3 = pout.tile([P, 3, b_c], f16, tag="of")
                nc.scalar.activation(of3[:], po3[:], AF.Copy, scale=1.0 / SW)
                nc.scalar.dma_start(oall_v[:, :, ot, :], of3[:])

            who0 = load_w(wo, 0)
            # prefill: ot0/ot1 contract heads 0-13 while the last heads'
            # softmax drains; the head-14/15 pair (kp=7) lands afterwards
            po3a = ps_s.tile([P, 3, b_c], f32, tag="sc")
            po3b = ps_s.tile([P, 3, b_c], f32, tag="sc")
            for po3_, oc_ in ((po3a, 0), (po3b, 1)):
                for kp in range(7):
                    for t in range(3):
                        wo_mm(po3_, who0, oc_, kp, t, False)
            pe_warm(36)
            first_grp = True
            for po3_, oc_ in ((po3a, 0), (po3b, 1)):
                for t in range(3):
                    wo_mm(po3_, who0, oc_, 7, t, True)
                    # the next attT(i) lands ~4.3us later; keep PE warm
                    if first_grp and t < 2:
                        pe_warm(16)
                wo_evict(po3_, oc_)
                first_grp = False
            for q in range(4):
                who = who0 if q == 0 else load_w(wo, q)
                for oc in range(2 if q == 0 else 0, 4):
                    ot = q * 4 + oc
                    if ot % 2 == 0:
                        po3 = ps_s.tile([P, 3, b_c], f32, tag="sc")
                        for kp in range(8):
                            for t in range(3):
                                wo_mm(po3, who, oc, kp, t, kp == 7)
                        wo_evict(po3, ot)
                    else:
                        # odd tiles run through the idle single-bank pool so
                        # the 3-bank score-psum rotation gets 2x the slack
                        of3 = pout.tile([P, 3, b_c], f16, tag="of")
                        for t in range(3):
                            po1 = ps_mm.tile([P, b_c], f32, tag="mm")
                            for kp in range(8):
                                nc.tensor.matmul(
                                    po1[:],
                                    who[:, 2 * kp : 2 * kp + 2, oc * P : (oc + 1) * P],
                                    attT[:, t, 2 * kp : 2 * kp + 2, :],
                                    start=(kp == 0),
                                    stop=(kp == 7),
                                    perf_mode=DR,
                                )
                            nc.scalar.activation(
                                of3[:, t, :], po1[:], AF.Copy, scale=1.0 / SW
                            )
                        nc.scalar.dma_start(oall_v[:, :, ot, :], of3[:])

    nc.compile()
    return nc


def _get_nc():
    key = "full"
    if key not in _compiled:
        _compiled[key] = _build()
    return _compiled[key]


def _tile_rows(a):
    """[D, N] -> [128, (D//128)*N] with row d at [d%128, (d//128)*N:...]."""
    dd, n = a.shape
    return np.ascontiguousarray(
        a.reshape(dd // P, P, n).transpose(1, 0, 2).reshape(P, (dd // P) * n)
    )


def kernel(
    x_token,
    lat_token,
    fdbk_token,
    W_gate_L,
    b_gate_L,
    W_gate_X,
    b_gate_X,
    W_q,
    W_k,
    W_v,
    W_o,
    v_a,
):
    import ml_dtypes
    from concourse.bass_utils import run_bass_kernel_spmd

    nc = _get_nc()
    f32 = np.float32
    e4 = ml_dtypes.float8_e4m3

    def prep_w(W):
        wt = np.asarray(W, f32).T * SW  # [in, out] = W.T
        wt = np.clip(wt, -240.0, 240.0)
        # -> [p=in%128, (quarter, k=in//128, out%512)]
        a = wt.reshape(KT, P, 4, 512).transpose(1, 2, 0, 3)
        return np.ascontiguousarray(a.reshape(P, 4 * KT * 512)).astype(e4)

    w8 = {
        "wgl": prep_w(W_gate_L),
        "wgx": prep_w(W_gate_X),
        "wq": prep_w(W_q),
        "wk": prep_w(W_k),
        "wv": prep_w(W_v),
        "wo": prep_w(W_o),
    }
    bglT = np.ascontiguousarray(np.asarray(b_gate_L, f32).reshape(KT, P).T)
    bgxT = np.ascontiguousarray(np.asarray(b_gate_X, f32).reshape(KT, P).T)
    va = np.asarray(v_a, f32).reshape(H, DH).T * SV  # [DH, H]
    vab = np.ascontiguousarray(
        np.repeat(va[:, :, None], P, axis=2).reshape(DH, H * P)
    ).astype(e4)

    tok_full = [
        np.asarray(t, f32).reshape(B, D) for t in (x_token, lat_token, fdbk_token)
    ]
    tokT16 = [(tok_full[t].T * ST).astype(np.float16) for t in range(2)]
    fdbkT8 = np.clip(tok_full[2].T * ST, -240.0, 240.0).astype(e4)

    in_maps = []
    for c in range(N_CORES):
        s = slice(c * B_C, (c + 1) * B_C)
        m = {f"tok{t}": _tile_rows(tokT16[t][:, s]) for t in range(2)}
        m["fdbk8"] = _tile_rows(fdbkT8[:, s])
        m.update(w8)
        m.update({"bglT": bglT, "bgxT": bgxT, "vab": vab})
        in_maps.append(m)

    res = run_bass_kernel_spmd(nc, in_maps, list(range(N_CORES))).results

    out = []
    for t in range(3):
        cores = []
        for c in range(N_CORES):
            r = res[c]["oall"].reshape(P, 3, KT, B_C)[:, t]
            cores.append(
                np.ascontiguousarray(r).transpose(1, 0, 2).reshape(D, B_C)
            )
        full = np.concatenate(cores, axis=1)  # [D, B]
        o = full.T.astype(f32) / SW + tok_full[t]
        out.append(o.reshape(B, 1, D))
    return tuple(out)
